# revision 5
# baseline (speedup 1.0000x reference)
"""Causal self-attention Trainium2 kernel (8-core SPMD).

Problem: x[2,2048,1024], causal mask, Wqkv[3072,1024], Wo[1024,1024], fp32.
  qkv = x @ Wqkv.T ; per-head causal softmax attention ; out = attn @ Wo.T

Sharding (data + tensor parallel, per the head dimension):
  core c -> batch b = c // 4, heads {4g..4g+3} with g = c % 4.
  Each core computes Q,K,V for its 4 heads (512 qk cols + 256 v cols of the
  projection), runs causal attention for those heads, and multiplies by the
  matching 256 columns of Wo, producing a partial [2048, 1024] output.
  Host sums the 4 partials per batch (the tensor-parallel reduction).

Kernel layout choices (per core):
  - x is passed pre-transposed (xT [1024, 2048]) so all projection matmuls
    contract over d with d on partitions; weights are passed pre-transposed
    the same way. fp32 in memory, fp32r in the PE (full speed at N>=256).
  - Q^T/K^T [64, 2048] per head; scores are computed TRANSPOSED
    (scoresT[k, q] blocks of [128, 512]) so the AV matmul needs no
    transposes: lhsT = V block (natural layout).
  - V is augmented with a ones column (65 cols/head): the AV matmul's
    partition 64 accumulates sum_k(exp) = the softmax denominator for free.
  - Causality at block granularity: strictly-upper blocks are skipped;
    diagonal-straddling blocks compute only the valid column range, with the
    128x128 diagonal sub-block masked via the real mask input (transposed
    diag tile) before exp.
  - Softmax normalization: reciprocal of the sums row, broadcast across 64
    partitions with a K=1 ones-matmul, then one DVE multiply.
"""

import numpy as np

S = 2048
D = 1024
DH = 64
B = 2
NCORES = 8
HPC = 4  # heads per core
QKC = 2 * HPC * DH  # 512 q+k projection columns per core
VC = HPC * DH  # 256 v columns per core
P = 128
KO = D // P  # 8 contraction tiles
NQ = S // 512  # 4 q-chunks of 512
NSC = S // P  # 16 s-chunks of 128

_cache = {}


def _build():
    import concourse.bacc as bacc
    import concourse.mybir as mybir
    import concourse.tile as tile

    F32 = mybir.dt.float32
    F32R = mybir.dt.float32r
    EXP = mybir.ActivationFunctionType.Exp
    MUL = mybir.AluOpType.mult
    ADD = mybir.AluOpType.add

    nc = bacc.Bacc()
    xT_d = nc.dram_tensor("xT", [D, S], F32R, kind="ExternalInput")
    wqkT_d = nc.dram_tensor("wqkT", [D, QKC], F32R, kind="ExternalInput")
    wvT_d = nc.dram_tensor("wvT", [D, VC], F32R, kind="ExternalInput")
    woT_d = nc.dram_tensor("woT", [VC, D], F32R, kind="ExternalInput")
    maskT_d = nc.dram_tensor("maskT", [P, P], F32, kind="ExternalInput")
    out_d = nc.dram_tensor("out", [S, D], F32, kind="ExternalOutput")

    def r(ap):
        return ap

    with tile.TileContext(nc) as tc:
        with (
            tc.tile_pool(name="persist", bufs=1) as persist,
            tc.tile_pool(name="sb_small", bufs=3) as sb_small,
            tc.tile_pool(name="sb_exp", bufs=4) as sb_exp,
            tc.tile_pool(name="sb_out", bufs=3) as sb_out,
        ):
            xT_sb = persist.tile([P, KO, S], F32R, tag="xT")
            wqkT_sb = persist.tile([P, KO, QKC], F32R, tag="wqkT")
            wvT_sb = persist.tile([P, KO, VC], F32R, tag="wvT")
            woT_sb = persist.tile([P, 2, D], F32R, tag="woT")
            maskT_ext = persist.tile([P, 512], F32, tag="maskT")
            ones_sb = persist.tile([1, DH], F32R, tag="ones")
            qkT_sb = persist.tile([P, 4, S], F32R, tag="qkT")
            v_sb = persist.tile([P, NSC, HPC, DH + 1], F32R, tag="v")
            attn_sb = persist.tile([P, 2, S], F32R, tag="attn")

            # --- input DMAs ---
            for ko in range(KO):
                nc.sync.dma_start(xT_sb[:, ko, :], xT_d[ko * P : (ko + 1) * P, :])
            nc.sync.dma_start(wqkT_sb[:], wqkT_d.rearrange("(ko p) c -> p ko c", p=P))
            nc.sync.dma_start(wvT_sb[:], wvT_d.rearrange("(ko p) c -> p ko c", p=P))
            nc.sync.dma_start(woT_sb[:], woT_d.rearrange("(ct p) e -> p ct e", p=P))
            nc.vector.memset(maskT_ext[:], 0.0)
            nc.sync.dma_start(maskT_ext[:, 0:P], maskT_d[:])
            ones_f32 = persist.tile([P, DH], F32, tag="ones_f32")
            nc.vector.memset(ones_f32[:], 1.0)
            nc.vector.tensor_copy(out=ones_sb[:], in_=ones_f32[0:1, :])
            nc.vector.tensor_copy(
                out=v_sb[:, :, :, DH],
                in_=ones_f32[:, 0 : NSC * HPC].rearrange("p (a b) -> p a b", a=NSC),
            )

            # --- QK projection: qkT[c, s] for the 512 local q+k columns ---
            with tc.tile_pool(name="pp_qk", bufs=3, space="PSUM") as pp_qk:
                for nn in range(NQ):
                    for mm in range(4):
                        ps = pp_qk.tile([P, 512], F32, tag="qk")
                        for ko in range(KO):
                            nc.tensor.matmul(
                                ps[:],
                                r(wqkT_sb[:, ko, mm * P : (mm + 1) * P]),
                                r(xT_sb[:, ko, nn * 512 : (nn + 1) * 512]),
                                start=(ko == 0),
                                stop=(ko == KO - 1),
                            )
                        nc.any.tensor_copy(
                            out=qkT_sb[:, mm, nn * 512 : (nn + 1) * 512], in_=ps[:]
                        )

                # --- V projection (natural [s, c] layout) ---
                for sc in range(NSC):
                    psv = pp_qk.tile([P, VC], F32, tag="v")
                    for ko in range(KO):
                        nc.tensor.matmul(
                            psv[:],
                            r(xT_sb[:, ko, sc * P : (sc + 1) * P]),
                            r(wvT_sb[:, ko, :]),
                            start=(ko == 0),
                            stop=(ko == KO - 1),
                        )
                    nc.any.tensor_copy(
                        out=v_sb[:, sc, :, 0:DH],
                        in_=psv.rearrange("p (h d) -> p h d", h=HPC),
                    )

            # --- attention + output projection, per 512-wide q chunk ---
            with (
                tc.tile_pool(name="pp_s", bufs=2, space="PSUM") as pp_s,
                tc.tile_pool(name="pp_av", bufs=2, space="PSUM") as pp_av,
                tc.tile_pool(name="pp_b", bufs=1, space="PSUM") as pp_b,
                tc.tile_pool(name="pp_o", bufs=2, space="PSUM") as pp_o,
            ):
                for qc in range(NQ):
                    nkb = 4 * qc + 4  # causal: k blocks 0 .. 4qc+3
                    for h in range(HPC):
                        hp = (h % 2) * DH  # partition base within the m-tile
                        mq = h // 2  # Q m-tile; K m-tile = 2 + h//2
                        ps_av = pp_av.tile([DH + 1, 512], F32, tag="av")
                        for kb in range(nkb):
                            m = kb - 4 * qc  # >= 0 on diagonal straddlers
                            off = max(0, m) * P
                            ps_s = pp_s.tile([P, 512], F32, tag="s")
                            exp_t = sb_exp.tile([P, 512], F32R, tag="exp")
                            nc.tensor.matmul(
                                ps_s[:, off:512],
                                r(qkT_sb[hp : hp + DH, 2 + mq, kb * P : (kb + 1) * P]),
                                r(qkT_sb[hp : hp + DH, mq, qc * 512 + off : (qc + 1) * 512]),
                                start=True,
                                stop=True,
                            )
                            if m >= 0:
                                # scale + mask (diag tile real mask, zeros tail)
                                nc.vector.scalar_tensor_tensor(
                                    out=ps_s[:, off:512],
                                    in0=ps_s[:, off:512],
                                    scalar=0.125,
                                    in1=maskT_ext[:, 0 : 512 - off],
                                    op0=MUL,
                                    op1=ADD,
                                )
                                nc.scalar.activation(exp_t[:, off:512], ps_s[:, off:512], EXP)
                            else:
                                nc.scalar.activation(
                                    exp_t[:, 0:512], ps_s[:, 0:512], EXP, scale=0.125
                                )
                            nc.tensor.matmul(
                                ps_av[:, off:512],
                                r(v_sb[:, kb, h, :]),
                                r(exp_t[:, off:512]),
                                start=(kb == 0),
                                stop=(kb == nkb - 1),
                                skip_group_check=True,
                            )
                        # normalize: out = av * (1/sums) broadcast over partitions
                        recip = sb_small.tile([1, 512], F32R, tag="recip")
                        with nc.allow_low_precision(reason="softmax recip feeds f32r matmul"):
                            nc.vector.reciprocal(recip[:], ps_av[DH : DH + 1, :])
                        ps_b = pp_b.tile([DH, 512], F32, tag="b")
                        nc.tensor.matmul(ps_b[:], r(ones_sb[:]), r(recip[:]), start=True, stop=True)
                        bc_sb = sb_small.tile([DH, 512], F32, tag="bc")
                        nc.scalar.copy(bc_sb[:], ps_b[:])
                        nc.vector.tensor_mul(
                            out=attn_sb[hp : hp + DH, h // 2, qc * 512 : (qc + 1) * 512],
                            in0=ps_av[0:DH, :],
                            in1=bc_sb[:],
                        )

                    # --- output projection for this q chunk ---
                    for si in range(4):
                        sc = qc * 4 + si
                        for en in range(2):
                            ps_o = pp_o.tile([P, 512], F32, tag="o")
                            for ct in range(2):
                                nc.tensor.matmul(
                                    ps_o[:],
                                    r(attn_sb[:, ct, sc * P : (sc + 1) * P]),
                                    r(woT_sb[:, ct, en * 512 : (en + 1) * 512]),
                                    start=(ct == 0),
                                    stop=(ct == 1),
                                )
                            o_sb = sb_out.tile([P, 512], F32, tag="osb")
                            nc.any.tensor_copy(out=o_sb[:], in_=ps_o[:])
                            nc.sync.dma_start(
                                out_d[sc * P : (sc + 1) * P, en * 512 : (en + 1) * 512],
                                o_sb[:],
                            )

    nc.compile()
    return nc


def _get_nc():
    if "nc" not in _cache:
        _cache["nc"] = _build()
    return _cache["nc"]


def _shard(x, mask, Wqkv, Wo):
    in_maps = []
    maskT = np.ascontiguousarray(mask[0, 0, :P, :P].T.astype(np.float32))
    for c in range(NCORES):
        b = c // 4
        g = c % 4
        heads = [4 * g + i for i in range(HPC)]
        q_rows = np.concatenate([np.arange(h * DH, (h + 1) * DH) for h in heads])
        k_rows = D + q_rows
        v_rows = 2 * D + q_rows
        qk_rows = np.concatenate([q_rows, k_rows])
        in_maps.append(
            {
                "xT": np.ascontiguousarray(x[b].T),
                "wqkT": np.ascontiguousarray(Wqkv[qk_rows, :].T),
                "wvT": np.ascontiguousarray(Wqkv[v_rows, :].T),
                "woT": np.ascontiguousarray(Wo[:, q_rows].T),
                "maskT": maskT,
            }
        )
    return in_maps


def kernel(x, mask, Wqkv, Wo, _trace=False):
    from concourse.bass_utils import run_bass_kernel_spmd

    x = np.asarray(x, dtype=np.float32)
    mask = np.asarray(mask, dtype=np.float32)
    Wqkv = np.asarray(Wqkv, dtype=np.float32)
    Wo = np.asarray(Wo, dtype=np.float32)

    nc = _get_nc()
    in_maps = _shard(x, mask, Wqkv, Wo)
    res = run_bass_kernel_spmd(nc, in_maps, core_ids=list(range(NCORES)), trace=_trace)
    _cache["last_result"] = res

    out = np.zeros((B, S, D), dtype=np.float32)
    for c in range(NCORES):
        out[c // 4] += res.results[c]["out"]
    return out


# revision 7
# speedup vs baseline: 1.2908x; 1.2908x over previous
"""Causal self-attention Trainium2 kernel (8-core SPMD).

Problem: x[2,2048,1024], causal mask, Wqkv[3072,1024], Wo[1024,1024], fp32.
  qkv = x @ Wqkv.T ; per-head causal softmax attention ; out = attn @ Wo.T

Sharding (data + tensor parallel, per the head dimension):
  core c -> batch b = c // 4, heads {4g..4g+3} with g = c % 4.
  Each core computes Q,K,V for its 4 heads (512 qk cols + 256 v cols of the
  projection), runs causal attention for those heads, and multiplies by the
  matching 256 columns of Wo, producing a partial [2048, 1024] output.
  Host sums the 4 partials per batch (the tensor-parallel reduction).

Kernel layout choices (per core):
  - x and the weight slices are passed pre-transposed so all projection
    matmuls contract over d with d on partitions. Matmul operands are bf16
    (PE 1 cyc/row; fp32/f32r stream 3-4x slower on this silicon), PSUM
    accumulation fp32.
  - Q^T/K^T [64, 2048] per head; scores are computed TRANSPOSED
    (scoresT[k, q] blocks of [128, 512]) so the AV matmul needs no
    transposes: lhsT = V block (natural layout). Head pairs share the PE
    array via partition-base row tiling (K=64 each).
  - V is augmented with a ones column (65 cols/head): the AV matmul's
    partition 64 accumulates sum_k(exp) = the softmax denominator for free.
  - Causality at block granularity: strictly-upper blocks are skipped;
    diagonal-straddling blocks compute only the valid column range, with the
    128x128 diagonal sub-block masked via the real mask input (transposed
    diag tile) before exp.
  - Softmax normalization: fast-approx reciprocal of the sums row (sums>=1,
    so no edge cases), broadcast across 64 partitions with a K=1
    ones-matmul, then one DVE multiply.
"""

import os

import numpy as np

S = 2048
D = 1024
DH = 64
B = 2
NCORES = 8
HPC = 4  # heads per core
QKC = 2 * HPC * DH  # 512 q+k projection columns per core
VC = HPC * DH  # 256 v columns per core
P = 128
KO = D // P  # 8 contraction tiles
NQ = S // 512  # 4 q-chunks of 512
NSC = S // P  # 16 s-chunks of 128

COMPUTE_DT = os.environ.get("ATTN_COMPUTE_DT", "bf16")  # bf16 | f32r

_cache = {}


def _np_compute_dt():
    if COMPUTE_DT == "bf16":
        import ml_dtypes

        return ml_dtypes.bfloat16
    return np.float32


def _build():
    import concourse.bacc as bacc
    import concourse.mybir as mybir
    import concourse.tile as tile

    F32 = mybir.dt.float32
    CDT = mybir.dt.bfloat16 if COMPUTE_DT == "bf16" else mybir.dt.float32r
    EXP = mybir.ActivationFunctionType.Exp
    MUL = mybir.AluOpType.mult
    ADD = mybir.AluOpType.add

    nc = bacc.Bacc()
    xT_d = nc.dram_tensor("xT", [D, S], CDT, kind="ExternalInput")
    wqkT_d = nc.dram_tensor("wqkT", [D, QKC], CDT, kind="ExternalInput")
    wvT_d = nc.dram_tensor("wvT", [D, VC], CDT, kind="ExternalInput")
    woT_d = nc.dram_tensor("woT", [VC, D], CDT, kind="ExternalInput")
    maskT_d = nc.dram_tensor("maskT", [P, P], F32, kind="ExternalInput")
    out_d = nc.dram_tensor("out", [S, D], F32, kind="ExternalOutput")

    with tile.TileContext(nc) as tc:
        with (
            tc.tile_pool(name="persist", bufs=1) as persist,
            tc.tile_pool(name="sb_small", bufs=3) as sb_small,
            tc.tile_pool(name="sb_exp", bufs=4) as sb_exp,
            tc.tile_pool(name="sb_out", bufs=3) as sb_out,
        ):
            xT_sb = persist.tile([P, KO, S], CDT, tag="xT")
            wqkT_sb = persist.tile([P, KO, QKC], CDT, tag="wqkT")
            wvT_sb = persist.tile([P, KO, VC], CDT, tag="wvT")
            woT_sb = persist.tile([P, 2, D], CDT, tag="woT")
            maskT_ext = persist.tile([P, 512], F32, tag="maskT")
            ones_sb = persist.tile([1, DH], CDT, tag="ones")
            qkT_sb = persist.tile([P, 4, S], CDT, tag="qkT")
            v_sb = persist.tile([P, NSC, HPC, DH + 1], CDT, tag="v")
            attn_sb = persist.tile([P, 2, S], CDT, tag="attn")

            # --- input DMAs ---
            for ko in range(KO):
                nc.sync.dma_start(xT_sb[:, ko, :], xT_d[ko * P : (ko + 1) * P, :])
            nc.sync.dma_start(wqkT_sb[:], wqkT_d.rearrange("(ko p) c -> p ko c", p=P))
            nc.sync.dma_start(wvT_sb[:], wvT_d.rearrange("(ko p) c -> p ko c", p=P))
            nc.sync.dma_start(woT_sb[:], woT_d.rearrange("(ct p) e -> p ct e", p=P))
            nc.vector.memset(maskT_ext[:], 0.0)
            nc.sync.dma_start(maskT_ext[:, 0:P], maskT_d[:])
            ones_f32 = persist.tile([P, DH], F32, tag="ones_f32")
            nc.vector.memset(ones_f32[:], 1.0)
            nc.vector.tensor_copy(out=ones_sb[:], in_=ones_f32[0:1, :])
            nc.vector.tensor_copy(
                out=v_sb[:, :, :, DH],
                in_=ones_f32[:, 0 : NSC * HPC].rearrange("p (a b) -> p a b", a=NSC),
            )

            # --- QK projection: qkT[c, s] for the 512 local q+k columns ---
            with tc.tile_pool(name="pp_qk", bufs=3, space="PSUM") as pp_qk:
                for nn in range(NQ):
                    for mm in range(4):
                        ps = pp_qk.tile([P, 512], F32, tag="qk")
                        for ko in range(KO):
                            nc.tensor.matmul(
                                ps[:],
                                wqkT_sb[:, ko, mm * P : (mm + 1) * P],
                                xT_sb[:, ko, nn * 512 : (nn + 1) * 512],
                                start=(ko == 0),
                                stop=(ko == KO - 1),
                            )
                        nc.vector.tensor_copy(
                            out=qkT_sb[:, mm, nn * 512 : (nn + 1) * 512], in_=ps[:]
                        )

                # --- V projection (natural [s, c] layout) ---
                for sc in range(NSC):
                    psv = pp_qk.tile([P, VC], F32, tag="v")
                    for ko in range(KO):
                        nc.tensor.matmul(
                            psv[:],
                            xT_sb[:, ko, sc * P : (sc + 1) * P],
                            wvT_sb[:, ko, :],
                            start=(ko == 0),
                            stop=(ko == KO - 1),
                        )
                    nc.vector.tensor_copy(
                        out=v_sb[:, sc, :, 0:DH],
                        in_=psv.rearrange("p (h d) -> p h d", h=HPC),
                    )

            # --- attention + output projection, per 512-wide q chunk ---
            with (
                tc.tile_pool(name="pp_s", bufs=2, space="PSUM") as pp_s,
                tc.tile_pool(name="pp_av", bufs=2, space="PSUM") as pp_av,
                tc.tile_pool(name="pp_b", bufs=1, space="PSUM") as pp_b,
                tc.tile_pool(name="pp_o", bufs=2, space="PSUM") as pp_o,
            ):
                for qc in range(NQ):
                    nkb = 4 * qc + 4  # causal: k blocks 0 .. 4qc+3
                    for h in range(HPC):
                        hp = (h % 2) * DH  # partition base within the m-tile
                        mq = h // 2  # Q m-tile; K m-tile = 2 + h//2
                        ps_av = pp_av.tile([DH + 1, 512], F32, tag="av")
                        for kb in range(nkb):
                            m = kb - 4 * qc  # >= 0 on diagonal straddlers
                            off = max(0, m) * P
                            ps_s = pp_s.tile([P, 512], F32, tag="s")
                            exp_t = sb_exp.tile([P, 512], CDT, tag="exp")
                            nc.tensor.matmul(
                                ps_s[:, off:512],
                                qkT_sb[hp : hp + DH, 2 + mq, kb * P : (kb + 1) * P],
                                qkT_sb[hp : hp + DH, mq, qc * 512 + off : (qc + 1) * 512],
                                start=True,
                                stop=True,
                            )
                            if m >= 0:
                                # scale + mask (diag tile real mask, zeros tail)
                                nc.vector.scalar_tensor_tensor(
                                    out=ps_s[:, off:512],
                                    in0=ps_s[:, off:512],
                                    scalar=0.125,
                                    in1=maskT_ext[:, 0 : 512 - off],
                                    op0=MUL,
                                    op1=ADD,
                                )
                                nc.scalar.activation(exp_t[:, off:512], ps_s[:, off:512], EXP)
                            else:
                                nc.scalar.activation(
                                    exp_t[:, 0:512], ps_s[:, 0:512], EXP, scale=0.125
                                )
                            nc.tensor.matmul(
                                ps_av[:, off:512],
                                v_sb[:, kb, h, :],
                                exp_t[:, off:512],
                                start=(kb == 0),
                                stop=(kb == nkb - 1),
                                skip_group_check=True,
                            )
                        # normalize: out = av * (1/sums) broadcast over partitions
                        sums_sb = sb_small.tile([1, 512], F32, tag="sums")
                        nc.vector.tensor_copy(out=sums_sb[:], in_=ps_av[DH : DH + 1, :])
                        recip_f = sb_small.tile([1, 512], F32, tag="recipf")
                        nc.vector.reciprocal_approx_fast(out=recip_f[:], in_=sums_sb[:])
                        recip = sb_small.tile([1, 512], CDT, tag="recip")
                        nc.vector.tensor_copy(out=recip[:], in_=recip_f[:])
                        ps_b = pp_b.tile([DH, 512], F32, tag="b")
                        nc.tensor.matmul(ps_b[:], ones_sb[:], recip[:], start=True, stop=True)
                        bc_sb = sb_small.tile([DH, 512], F32, tag="bc")
                        nc.scalar.copy(bc_sb[:], ps_b[:])
                        nc.vector.tensor_mul(
                            out=attn_sb[hp : hp + DH, h // 2, qc * 512 : (qc + 1) * 512],
                            in0=ps_av[0:DH, :],
                            in1=bc_sb[:],
                        )

                    # --- output projection for this q chunk ---
                    for si in range(4):
                        sc = qc * 4 + si
                        for en in range(2):
                            ps_o = pp_o.tile([P, 512], F32, tag="o")
                            for ct in range(2):
                                nc.tensor.matmul(
                                    ps_o[:],
                                    attn_sb[:, ct, sc * P : (sc + 1) * P],
                                    woT_sb[:, ct, en * 512 : (en + 1) * 512],
                                    start=(ct == 0),
                                    stop=(ct == 1),
                                )
                            o_sb = sb_out.tile([P, 512], F32, tag="osb")
                            nc.any.tensor_copy(out=o_sb[:], in_=ps_o[:])
                            nc.sync.dma_start(
                                out_d[sc * P : (sc + 1) * P, en * 512 : (en + 1) * 512],
                                o_sb[:],
                            )

    nc.compile()
    return nc


def _get_nc():
    if "nc" not in _cache:
        _cache["nc"] = _build()
    return _cache["nc"]


def _shard(x, mask, Wqkv, Wo):
    cdt = _np_compute_dt()
    in_maps = []
    maskT = np.ascontiguousarray(mask[0, 0, :P, :P].T.astype(np.float32))
    for c in range(NCORES):
        b = c // 4
        g = c % 4
        heads = [4 * g + i for i in range(HPC)]
        q_rows = np.concatenate([np.arange(h * DH, (h + 1) * DH) for h in heads])
        k_rows = D + q_rows
        v_rows = 2 * D + q_rows
        qk_rows = np.concatenate([q_rows, k_rows])
        in_maps.append(
            {
                "xT": np.ascontiguousarray(x[b].T.astype(cdt)),
                "wqkT": np.ascontiguousarray(Wqkv[qk_rows, :].T.astype(cdt)),
                "wvT": np.ascontiguousarray(Wqkv[v_rows, :].T.astype(cdt)),
                "woT": np.ascontiguousarray(Wo[:, q_rows].T.astype(cdt)),
                "maskT": maskT,
            }
        )
    return in_maps


def kernel(x, mask, Wqkv, Wo, _trace=False):
    from concourse.bass_utils import run_bass_kernel_spmd

    x = np.asarray(x, dtype=np.float32)
    mask = np.asarray(mask, dtype=np.float32)
    Wqkv = np.asarray(Wqkv, dtype=np.float32)
    Wo = np.asarray(Wo, dtype=np.float32)

    nc = _get_nc()
    in_maps = _shard(x, mask, Wqkv, Wo)
    res = run_bass_kernel_spmd(nc, in_maps, core_ids=list(range(NCORES)), trace=_trace)
    _cache["last_result"] = res

    out = np.zeros((B, S, D), dtype=np.float32)
    for c in range(NCORES):
        out[c // 4] += res.results[c]["out"]
    return out


# revision 10
# speedup vs baseline: 1.8052x; 1.3986x over previous
"""Causal self-attention Trainium2 kernel (8-core SPMD).

Problem: x[2,2048,1024], causal mask, Wqkv[3072,1024], Wo[1024,1024], fp32.
  qkv = x @ Wqkv.T ; per-head causal softmax attention ; out = attn @ Wo.T

Sharding (data + tensor parallel, per the head dimension):
  core c -> batch b = c // 4, heads {4g..4g+3} with g = c % 4.
  Each core computes Q,K,V for its 4 heads (512 qk cols + 256 v cols of the
  projection), runs causal attention for those heads, and multiplies by the
  matching 256 columns of Wo, producing a partial [2048, 1024] output.
  Host sums the 4 partials per batch (the tensor-parallel reduction).

Kernel structure (per core):
  - bf16 matmul operands (PE 1 cyc/row), fp32 PSUM accumulation. Inputs are
    host-transposed/cast so every matmul contracts with the right layout.
  - Projection is emitted in sequence chunks interleaved with attention:
    attention for q-chunk qc only needs projection chunks nn <= qc, so ACT
    softmax-exp work overlaps PE projection matmuls.
  - Scores are computed TRANSPOSED (scoresT[k, q] blocks, head pairs packed
    in the PE via partition-base row tiling) so AV needs no transposes.
    Per (head, q-chunk): all score blocks + exps are issued first into a
    deep exp-tile pool, then all AV matmuls, so the PE never sits behind a
    single exp on the critical path.
  - Causality: strictly-upper blocks skipped; diagonal straddlers compute
    only the valid columns; the 128x128 diagonal sub-block is exp'd
    unmasked and then multiplied by a binary mask tile (derived from the
    real mask input on host).
  - V carries a ones column (65 cols/head): AV's partition 64 accumulates
    the softmax denominator for free. Normalization = fast-approx
    reciprocal (sums >= 1) broadcast over partitions via a K=1 ones-matmul.
"""

import os

import numpy as np

S = 2048
D = 1024
DH = 64
B = 2
NCORES = 8
HPC = 4  # heads per core
QKC = 2 * HPC * DH  # 512 q+k projection columns per core
VC = HPC * DH  # 256 v columns per core
P = 128
KO = D // P  # 8 contraction tiles
NQ = S // 512  # 4 q-chunks of 512
NSC = S // P  # 16 s-chunks of 128

COMPUTE_DT = os.environ.get("ATTN_COMPUTE_DT", "bf16")  # bf16 | f32r

_cache = {}


def _np_compute_dt():
    if COMPUTE_DT == "bf16":
        import ml_dtypes

        return ml_dtypes.bfloat16
    return np.float32


def _build():
    import concourse.bacc as bacc
    import concourse.mybir as mybir
    import concourse.tile as tile

    F32 = mybir.dt.float32
    CDT = mybir.dt.bfloat16 if COMPUTE_DT == "bf16" else mybir.dt.float32r
    EXP = mybir.ActivationFunctionType.Exp

    nc = bacc.Bacc()
    xT_d = nc.dram_tensor("xT", [D, S], CDT, kind="ExternalInput")
    wqkT_d = nc.dram_tensor("wqkT", [D, QKC], CDT, kind="ExternalInput")
    wvT_d = nc.dram_tensor("wvT", [D, VC], CDT, kind="ExternalInput")
    woT_d = nc.dram_tensor("woT", [VC, D], CDT, kind="ExternalInput")
    maskT_d = nc.dram_tensor("maskT", [P, P], CDT, kind="ExternalInput")
    out_d = nc.dram_tensor("out", [S, D], F32, kind="ExternalOutput")

    with tile.TileContext(nc) as tc:
        with (
            tc.tile_pool(name="persist", bufs=1) as persist,
            tc.tile_pool(name="sb_small", bufs=3) as sb_small,
            tc.tile_pool(name="sb_exp", bufs=18) as sb_exp,
            tc.tile_pool(name="sb_out", bufs=3) as sb_out,
            tc.tile_pool(name="pp_s", bufs=3, space="PSUM") as pp_s,
            tc.tile_pool(name="pp_av", bufs=2, space="PSUM") as pp_av,
            tc.tile_pool(name="pp_b", bufs=1, space="PSUM") as pp_b,
            tc.tile_pool(name="pp_o", bufs=2, space="PSUM") as pp_o,
        ):
            xT_sb = persist.tile([P, KO, S], CDT, tag="xT")
            wqkT_sb = persist.tile([P, KO, QKC], CDT, tag="wqkT")
            wvT_sb = persist.tile([P, KO, VC], CDT, tag="wvT")
            woT_sb = persist.tile([P, 2, D], CDT, tag="woT")
            maskT_sb = persist.tile([P, P], CDT, tag="maskT")
            ones_sb = persist.tile([1, DH], CDT, tag="ones")
            qkT_sb = persist.tile([P, 4, S], CDT, tag="qkT")
            v_sb = persist.tile([P, NSC, HPC, DH + 1], CDT, tag="v")
            attn_sb = persist.tile([P, 2, S], CDT, tag="attn")

            # --- input DMAs: weights+x split across two queues, per-ko ---
            for ko in range(KO):
                eng = nc.sync if ko % 2 == 0 else nc.gpsimd
                eng.dma_start(
                    wqkT_sb[:, ko, :],
                    wqkT_d[ko * P : (ko + 1) * P, :],
                )
            for ko in range(KO):
                eng = nc.gpsimd if ko % 2 == 0 else nc.sync
                eng.dma_start(xT_sb[:, ko, :], xT_d[ko * P : (ko + 1) * P, :])
            for ko in range(KO):
                eng = nc.sync if ko % 2 == 0 else nc.gpsimd
                eng.dma_start(wvT_sb[:, ko, :], wvT_d[ko * P : (ko + 1) * P, :])
            nc.sync.dma_start(maskT_sb[:], maskT_d[:])
            nc.gpsimd.dma_start(woT_sb[:], woT_d.rearrange("(ct p) e -> p ct e", p=P))
            ones_f32 = persist.tile([P, DH], F32, tag="ones_f32")
            nc.vector.memset(ones_f32[:], 1.0)
            nc.vector.tensor_copy(out=ones_sb[:], in_=ones_f32[0:1, :])
            nc.vector.tensor_copy(
                out=v_sb[:, :, :, DH],
                in_=ones_f32[:, 0 : NSC * HPC].rearrange("p (a b) -> p a b", a=NSC),
            )

            for qc in range(NQ):
                # --- projection chunk nn = qc (s columns qc*512..qc*512+512) ---
                for mm in range(4):
                    ps = pp_s.tile([P, 512], F32, tag="s")
                    for ko in range(KO):
                        nc.tensor.matmul(
                            ps[:],
                            wqkT_sb[:, ko, mm * P : (mm + 1) * P],
                            xT_sb[:, ko, qc * 512 : (qc + 1) * 512],
                            start=(ko == 0),
                            stop=(ko == KO - 1),
                        )
                    nc.vector.tensor_copy(
                        out=qkT_sb[:, mm, qc * 512 : (qc + 1) * 512], in_=ps[:]
                    )
                for sc in range(4 * qc, 4 * qc + 4):
                    psv_full = pp_o.tile([P, 512], F32, tag="o", name="psv")
                    psv = psv_full[:, :VC]
                    for ko in range(KO):
                        nc.tensor.matmul(
                            psv[:],
                            xT_sb[:, ko, sc * P : (sc + 1) * P],
                            wvT_sb[:, ko, :],
                            start=(ko == 0),
                            stop=(ko == KO - 1),
                        )
                    nc.vector.tensor_copy(
                        out=v_sb[:, sc, :, 0:DH],
                        in_=psv.rearrange("p (h d) -> p h d", h=HPC),
                    )

                # --- attention for q-chunk qc ---
                nkb = 4 * qc + 4  # causal: k blocks 0 .. 4qc+3
                for h in range(HPC):
                    hp = (h % 2) * DH  # partition base within the m-tile
                    mq = h // 2  # Q m-tile; K m-tile = 2 + h//2
                    exps = []
                    for kb in range(nkb):
                        m = kb - 4 * qc  # >= 0 on diagonal straddlers
                        off = max(0, m) * P
                        ps_s = pp_s.tile([P, 512], F32, tag="s")
                        exp_t = sb_exp.tile([P, 512], CDT, tag="exp")
                        nc.tensor.matmul(
                            ps_s[:, off:512],
                            qkT_sb[hp : hp + DH, 2 + mq, kb * P : (kb + 1) * P],
                            qkT_sb[hp : hp + DH, mq, qc * 512 + off : (qc + 1) * 512],
                            start=True,
                            stop=True,
                        )
                        nc.scalar.activation(
                            exp_t[:, off:512], ps_s[:, off:512], EXP, scale=0.125
                        )
                        if m >= 0:
                            # zero the above-diagonal part of the 128x128
                            # diagonal sub-block (binary mask from input)
                            nc.vector.tensor_mul(
                                out=exp_t[:, off : off + P],
                                in0=exp_t[:, off : off + P],
                                in1=maskT_sb[:],
                            )
                        exps.append((exp_t, off))
                    ps_av = pp_av.tile([DH + 1, 512], F32, tag="av")
                    for kb, (exp_t, off) in enumerate(exps):
                        nc.tensor.matmul(
                            ps_av[:, off:512],
                            v_sb[:, kb, h, :],
                            exp_t[:, off:512],
                            start=(kb == 0),
                            stop=(kb == nkb - 1),
                            skip_group_check=True,
                        )
                    # normalize: out = av * (1/sums) broadcast over partitions
                    sums_sb = sb_small.tile([1, 512], F32, tag="sums")
                    nc.vector.tensor_copy(out=sums_sb[:], in_=ps_av[DH : DH + 1, :])
                    recip_f = sb_small.tile([1, 512], F32, tag="recipf")
                    nc.vector.reciprocal_approx_fast(out=recip_f[:], in_=sums_sb[:])
                    recip = sb_small.tile([1, 512], CDT, tag="recip")
                    nc.vector.tensor_copy(out=recip[:], in_=recip_f[:])
                    ps_b = pp_b.tile([DH, 512], F32, tag="b")
                    nc.tensor.matmul(ps_b[:], ones_sb[:], recip[:], start=True, stop=True)
                    bc_sb = sb_small.tile([DH, 512], F32, tag="bc")
                    nc.vector.tensor_copy(out=bc_sb[:], in_=ps_b[:])
                    nc.vector.tensor_mul(
                        out=attn_sb[hp : hp + DH, h // 2, qc * 512 : (qc + 1) * 512],
                        in0=ps_av[0:DH, :],
                        in1=bc_sb[:],
                    )

                # --- output projection for this q chunk ---
                for si in range(4):
                    sc = qc * 4 + si
                    for en in range(2):
                        ps_o = pp_o.tile([P, 512], F32, tag="o")
                        for ct in range(2):
                            nc.tensor.matmul(
                                ps_o[:],
                                attn_sb[:, ct, sc * P : (sc + 1) * P],
                                woT_sb[:, ct, en * 512 : (en + 1) * 512],
                                start=(ct == 0),
                                stop=(ct == 1),
                            )
                        o_sb = sb_out.tile([P, 512], F32, tag="osb")
                        nc.vector.tensor_copy(out=o_sb[:], in_=ps_o[:])
                        nc.sync.dma_start(
                            out_d[sc * P : (sc + 1) * P, en * 512 : (en + 1) * 512],
                            o_sb[:],
                        )

    nc.compile()
    return nc


def _get_nc():
    if "nc" not in _cache:
        _cache["nc"] = _build()
    return _cache["nc"]


def _shard(x, mask, Wqkv, Wo):
    cdt = _np_compute_dt()
    in_maps = []
    # binary mask for the transposed 128x128 diagonal block:
    # valid (mask==0) -> 1.0, masked (-inf/large-negative) -> 0.0
    maskT = np.ascontiguousarray((mask[0, 0, :P, :P].T >= 0).astype(cdt))
    for c in range(NCORES):
        b = c // 4
        g = c % 4
        heads = [4 * g + i for i in range(HPC)]
        q_rows = np.concatenate([np.arange(h * DH, (h + 1) * DH) for h in heads])
        k_rows = D + q_rows
        v_rows = 2 * D + q_rows
        qk_rows = np.concatenate([q_rows, k_rows])
        in_maps.append(
            {
                "xT": np.ascontiguousarray(x[b].T.astype(cdt)),
                "wqkT": np.ascontiguousarray(Wqkv[qk_rows, :].T.astype(cdt)),
                "wvT": np.ascontiguousarray(Wqkv[v_rows, :].T.astype(cdt)),
                "woT": np.ascontiguousarray(Wo[:, q_rows].T.astype(cdt)),
                "maskT": maskT,
            }
        )
    return in_maps


def kernel(x, mask, Wqkv, Wo, _trace=False):
    from concourse.bass_utils import run_bass_kernel_spmd

    x = np.asarray(x, dtype=np.float32)
    mask = np.asarray(mask, dtype=np.float32)
    Wqkv = np.asarray(Wqkv, dtype=np.float32)
    Wo = np.asarray(Wo, dtype=np.float32)

    nc = _get_nc()
    in_maps = _shard(x, mask, Wqkv, Wo)
    res = run_bass_kernel_spmd(nc, in_maps, core_ids=list(range(NCORES)), trace=_trace)
    _cache["last_result"] = res

    out = np.zeros((B, S, D), dtype=np.float32)
    for c in range(NCORES):
        out[c // 4] += res.results[c]["out"]
    return out
